# revision 24
# baseline (speedup 1.0000x reference)
"""Trainium2 Bass kernel for CustomRNN:
    h = tanh(x @ W1 + b1)                         [B,T,U]
    y_t = h_t + tanh(y_{t-1} @ W2 + b2)           (scan over T, y_{-1} = 0)

Strategy (8 NeuronCores, data-parallel over batch B=256 -> 32 rows/core):
  * Everything on-chip lives TRANSPOSED: state/AT/hT tiles are [u (part), batch (free)]
    so the sequential scan's matmuls need no per-step transposes.
  * Split-state trick: y_t = h_t + A_t with A_t = tanh(z_t + b2),
    z_{t+1} = h_t@W2 + A_t@W2.  The h_t@W2 matmuls run while the tanh of the
    previous step is still in flight (they only need h, ready long ago); only
    the A_t@W2 matmuls + tanh are on the serial critical path.  The DVE add
    (y = h + A) is output-only, fully off the chain.
  * One PSUM accumulation group per scan step (single start=True): the
    per-element has_written bit makes each region's first matmul an
    overwrite, so b2/h/A matmuls can be ordered freely for weight-load reuse.
  * Scan + phase-1 matmuls in fp16 (full PE rate, fast weight load), fp32
    PSUM accumulation, fp32 output assembly.
  * Group-local column layouts are b-major so DRAM transfers get
    GR*U*4 = 16KB contiguous runs per batch row.
  * Phase 1 (x@W1, 2-group lead) and phase 3 (transpose-back + DMA out,
    2-group lag; y DMAs on the idle Pool/SWDGE queue) interleave with the
    scan and hide under its chain latency.
"""

import numpy as np

import concourse.bacc as bacc
import concourse.bass as bass
import concourse.mybir as mybir
import concourse.tile as tile
from concourse import bass_utils

F32 = mybir.dt.float32
F16 = mybir.dt.float16

B, T, D, U = 256, 512, 256, 256
NCORES = 8
BS = B // NCORES   # 32 batch rows per core
GR = 16            # scan steps per group
P = 128
NB = (BS * GR) // P   # 128-row blocks per group (4)
BHB = P // GR         # batch rows per block (8)


def build_rnn(T_steps=T, scan_dt=F16, use_b2=True):
    assert T_steps % GR == 0
    NG = T_steps // GR
    TB = GR * BS  # free columns per group (512), col = b*GR + t_local

    nc = bacc.Bacc("TRN2", debug=False)

    x = nc.dram_tensor("x", (BS, T_steps, D), F32, kind="ExternalInput")
    W1 = nc.dram_tensor("W1", (D, U), F32, kind="ExternalInput")
    b1 = nc.dram_tensor("b1", (U,), F32, kind="ExternalInput")
    W2 = nc.dram_tensor("W2", (U, U), F32, kind="ExternalInput")
    b2 = nc.dram_tensor("b2", (U,), F32, kind="ExternalInput")
    y = nc.dram_tensor("y", (BS, T_steps, U), F32, kind="ExternalOutput")

    ident_dram = nc.inline_tensor(np.eye(P, dtype=np.float32), "ident")
    ones_dram = nc.inline_tensor(np.ones((1, BS), dtype=np.float16), "ones_row")

    x_v = x.ap().rearrange("b (g t) u -> g b t u", t=GR)
    y_v = y.ap().rearrange("b (g t) u -> g b t u", t=GR)

    with tile.TileContext(nc) as tc:
        with (
            tc.tile_pool(name="const", bufs=1) as cpool,
            tc.tile_pool(name="xin", bufs=2) as xp,
            tc.tile_pool(name="xT", bufs=3) as xtp,
            tc.tile_pool(name="hT", bufs=3) as hTp,
            tc.tile_pool(name="AT", bufs=3) as atp,
            tc.tile_pool(name="yT", bufs=4) as yTp,
            tc.tile_pool(name="ynat", bufs=2) as ynp,
            tc.tile_pool(name="ph", bufs=3, space="PSUM") as php,
            tc.tile_pool(name="pz", bufs=3, space="PSUM") as pzp,
            tc.tile_pool(name="ptr", bufs=2, space="PSUM") as ptp,
        ):
            # ---- constants ----
            W1s = cpool.tile([P, 2, U], scan_dt, tag="W1s")
            nc.gpsimd.dma_start(W1s, W1.ap().rearrange("(c p) u -> p c u", p=P))
            W2s = cpool.tile([P, 2, U], scan_dt, tag="W2s")
            nc.gpsimd.dma_start(W2s, W2.ap().rearrange("(c p) u -> p c u", p=P))
            b1s = cpool.tile([P, 2], F32, tag="b1s")
            nc.sync.dma_start(b1s, b1.ap().rearrange("(c p) -> p c", p=P))
            b2s = cpool.tile([1, U], scan_dt, tag="b2s")
            nc.gpsimd.dma_start(b2s, b2.ap().rearrange("(a u) -> a u", a=1))
            ones_t = cpool.tile([1, BS], scan_dt, tag="ones")
            nc.sync.dma_start(ones_t, ones_dram.ap())
            ident = cpool.tile([P, P], F32, tag="ident")
            nc.sync.dma_start(ident, ident_dram.ap())

            st = [dict() for _ in range(NG)]

            # Background work (phase 1 / phase 3) is emitted through a
            # cost-budgeted queue: at most ~one ACT-idle-window's worth of PE
            # work, one DMA, and one ACT op is inserted per scan step, so
            # background ops never pile up between a tanh and the A-matmuls
            # in an engine FIFO.  Items are (kind, cost, closure); order is
            # preserved across kinds.
            def phase1_work(g):
                """Work items for group g's phase 1 (load lead handled by caller)."""
                s = st[g]
                items = []

                def xin_dma(blk):
                    def run():
                        if blk == 0:
                            s["xin"] = xp.tile([P, NB, D], F32, tag="xin", name="xin")
                        nc.sync.dma_start(
                            s["xin"][:, blk, :], x_v[g, blk * BHB:(blk + 1) * BHB])
                    return run

                for blk in range(NB):
                    items.append(("dma", 1, xin_dma(blk)))

                def xpose(blk, dc):
                    def run():
                        if blk == 0 and dc == 0:
                            s["xT"] = xtp.tile([P, 2, TB], scan_dt, tag="xT", name="xT")
                        pt = ptp.tile([P, P], F32, tag="ptr", name="ptr")
                        nc.tensor.transpose(
                            pt, s["xin"][:, blk, dc * P:(dc + 1) * P], ident)
                        nc.vector.tensor_copy(
                            out=s["xT"][:, dc, blk * P:(blk + 1) * P], in_=pt)
                    return run

                for blk in range(NB):
                    for dc in (0, 1):
                        items.append(("pe", 107, xpose(blk, dc)))
                return items

            def phase1_mm_work(g):
                s = st[g]
                items = []

                def h_mm(uc, dc):
                    def run():
                        if uc == 0 and dc == 0:
                            s["hT"] = hTp.tile([P, 2, TB], scan_dt, tag="hT", name="hT")
                            s["ph"] = [None, None]
                        if dc == 0:
                            s["ph"][uc] = php.tile([P, TB], F32, tag="ph", name="ph")
                        nc.tensor.matmul(
                            s["ph"][uc],
                            W1s[:, dc, uc * P:(uc + 1) * P],
                            s["xT"][:, dc, :],
                            start=(dc == 0), stop=(dc == 1),
                        )
                    return run

                def h_act(uc, hh):
                    HH = TB // 2
                    def run():
                        nc.scalar.activation(
                            s["hT"][:, uc, hh * HH:(hh + 1) * HH],
                            s["ph"][uc][:, hh * HH:(hh + 1) * HH],
                            mybir.ActivationFunctionType.Tanh,
                            bias=b1s[:, uc:uc + 1],
                        )
                    return run

                for uc in (0, 1):
                    for dc in (0, 1):
                        items.append(("pe", 213, h_mm(uc, dc)))
                    for hh in (0, 1):
                        items.append(("act", 398, h_act(uc, hh)))
                return items

            def output_work(g):
                s = st[g]
                items = []

                def ypose(blk, uc):
                    def run():
                        if blk == 0 and uc == 0:
                            s["ynat"] = ynp.tile([P, NB, U], F32, tag="ynat", name="ynat")
                        pt = ptp.tile([P, P], F32, tag="ptr", name="ptr")
                        nc.tensor.transpose(
                            pt, s["yT"][:, uc, blk * P:(blk + 1) * P], ident)
                        nc.vector.tensor_copy(
                            out=s["ynat"][:, blk, uc * P:(uc + 1) * P], in_=pt)
                    return run

                def y_dma(blk):
                    def run():
                        nc.sync.dma_start(
                            y_v[g, blk * BHB:(blk + 1) * BHB], s["ynat"][:, blk, :])
                    return run

                for blk in range(NB):
                    items.append(("pe", 107, ypose(blk, 0)))
                    items.append(("pe", 107, ypose(blk, 1)))
                    items.append(("dma", 1, y_dma(blk)))
                return items

            from collections import deque
            work = deque()

            # ---------- scan ----------
            AT_prev = [None]

            def scan_step(t):
                g, j = divmod(t, GR)
                s = st[g]
                if j == 0:
                    s["yT"] = yTp.tile([P, 2, TB], F32, tag="yT", name="yT")
                if t == 0 and not use_b2:
                    AT = atp.tile([P, 2, BS], scan_dt, tag="AT", name="AT")
                    nc.vector.memzero(AT)
                else:
                    ps = pzp.tile([P, 2, BS], F32, tag="pz", name="pz")
                    first = [True]

                    def mm(mc, w, rhs, stop=False):
                        nc.tensor.matmul(ps[:, mc, :], w, rhs,
                                         start=first[0], stop=stop)
                        first[0] = False

                    if use_b2:
                        mm(0, b2s[:, 0:P], ones_t)
                        mm(1, b2s[:, P:U], ones_t, stop=(t == 0))
                    if t > 0:
                        gp, jp = divmod(t - 1, GR)
                        hprev = st[gp]["hT"]
                        h_rhs = [hprev[:, kc, :].rearrange(
                            "p (b j) -> p b j", j=GR)[:, :, jp] for kc in (0, 1)]
                        # h-matmuls: no dependency on this step's tanh -> they
                        # execute while ACT_t runs.  A-matmuls go last; the
                        # h-matmuls run in reverse chunk order so the first
                        # A-matmul reuses the stationary weights already loaded.
                        for mc, kc in ((1, 1), (1, 0), (0, 1), (0, 0)):
                            mm(mc, W2s[:, kc, mc * P:(mc + 1) * P], h_rhs[kc])
                        for mc, kc in ((0, 0), (0, 1), (1, 0), (1, 1)):
                            mm(mc, W2s[:, kc, mc * P:(mc + 1) * P],
                               AT_prev[0][:, kc, :],
                               stop=(mc == 1 and kc == 1))
                    AT = atp.tile([P, 2, BS], scan_dt, tag="AT", name="AT")
                    nc.scalar.activation(AT, ps, mybir.ActivationFunctionType.Tanh)
                AT_prev[0] = AT
                # y_t = h_t + A_t (output only, off the chain)
                for uc in (0, 1):
                    nc.vector.tensor_add(
                        out=s["yT"][:, uc, :].rearrange(
                            "p (b j) -> p b j", j=GR)[:, :, j],
                        in0=s["hT"][:, uc, :].rearrange(
                            "p (b j) -> p b j", j=GR)[:, :, j],
                        in1=AT[:, uc, :],
                    )

            # ---------- emission ----------
            # prologue: phase 1 for groups 0 and 1 runs before the scan
            for it in phase1_work(0) + phase1_mm_work(0):
                it[2]()
            if NG > 1:
                for it in phase1_work(1) + phase1_mm_work(1):
                    it[2]()

            pe_cr = dma_cr = act_cr = 0.0
            for t in range(T_steps):
                g, j = divmod(t, GR)
                if j == 0:
                    if g + 2 < NG:
                        work.extend(phase1_work(g + 2))
                    if g + 1 < NG and g + 1 >= 2:
                        work.extend(phase1_mm_work(g + 1))
                    if g >= 2:
                        work.extend(output_work(g - 2))
                scan_step(t)
                pe_cr = min(pe_cr + 230, 500)
                dma_cr = min(dma_cr + 1, 2)
                act_cr = min(act_cr + 220, 600)
                while work:
                    kind, cost, run = work[0]
                    if kind == "pe":
                        if pe_cr < cost:
                            break
                        pe_cr -= cost
                    elif kind == "dma":
                        if dma_cr < cost:
                            break
                        dma_cr -= cost
                    else:
                        if act_cr < cost:
                            break
                        act_cr -= cost
                    work.popleft()
                    run()
            while work:
                work.popleft()[2]()
            for g in range(max(NG - 2, 0), NG):
                for it in output_work(g):
                    it[2]()

    nc.finalize()
    return nc


_NC_CACHE = {}


def _get_nc(T_steps=T, use_b2=True):
    key = (T_steps, use_b2)
    if key not in _NC_CACHE:
        _NC_CACHE[key] = build_rnn(T_steps, use_b2=use_b2)
    return _NC_CACHE[key]


def kernel(x, W1, b1, W2, b2):
    b2 = np.asarray(b2, dtype=np.float32)
    use_b2 = bool(np.any(b2))
    nc = _get_nc(x.shape[1], use_b2)
    x = np.ascontiguousarray(x, dtype=np.float32)
    in_maps = []
    for c in range(NCORES):
        in_maps.append({
            "x": x[c * BS:(c + 1) * BS],
            "W1": np.asarray(W1, dtype=np.float32),
            "b1": np.asarray(b1, dtype=np.float32),
            "W2": np.asarray(W2, dtype=np.float32),
            "b2": b2,
        })
    res = bass_utils.run_bass_kernel_spmd(nc, in_maps, core_ids=list(range(NCORES)))
    return np.concatenate([r["y"] for r in res.results], axis=0)
